# revision 15
# baseline (speedup 1.0000x reference)
"""Multi-head masked self-attention on 8 trn2 NeuronCores.

Problem: B=2, T=2048, H=1024, nH=16 heads (head_dim=64), causal softmax
attention with QKV projections; scores scaled by 1/sqrt(H).

Sharding: heads across cores (2 heads per core), both batches on every
core. QKV weights column-sharded by head: core m gets rows
[128m, 128m+128) of each projection matrix.

Per-core device program (mixed precision, tolerance 2e-2):

  x^T [1024, 2048] streamed in bf16 -> SBUF [128, 8cb, 2048]
  Q^T/K^T [128(2h*64d), T] = Wn^T @ x^T in bf16 (PE, 8-block f32 PSUM
      accumulation); DVE evicts with per-partition bias add straight to
      fp8(e4m3) tiles laid out [128, 2, T] whose second contraction tile
      is persistent zeros (DoubleRow zero-padding).
  V^T likewise but evicted to bf16 per-head tiles vt_h [96, T] (head1
      via cross-partition DVE copy 64:128 -> 0:64); row 64 = ones.
      One XBAR dma_start_transpose per head builds V' [128k, 16kb, 96]
      with column 64 = ones (softmax denominator column).
  Scores (transposed): S^T[k, q] fp8 DoubleRow matmuls (0.5 cyc/row),
      contraction [64, 2, .] zero-padded, q chunks of <=256.
      ACT evicts exp(S/32) -> bf16 P tiles [128, 2, 512]; causal mask
      applied post-exp as gpsimd affine_select zeroing on the diagonal
      128-col block (both heads in one op).
  O'^T [65, 512] = sum_kb V'[:, kb, :65].T @ P^T (bf16 PE accumulation;
      row 64 accumulates Z). DVE evicts psO -> SBUF f32, DMA to DRAM
      out [B, 2, 65, T]. Host divides by Z and transposes (cheap).

  Emission is software-pipelined: batch b1's projections are emitted
  interleaved with b0's attention q-tiles (attention is ACT/exp-bound,
  projections are PE-bound), and b0's projections for the next rep
  overlap b1's attention.
"""
import sys

sys.path.insert(0, "/opt/trn_rl_repo")

import numpy as np
import ml_dtypes

B = 2
T = 2048
H = 1024
NHEADS = 16
HD = 64
NCORES = 8
P = 128
CB = H // P            # 8 contraction blocks for projections
QTILE = 512
NQT = T // QTILE       # 4 q-tiles
NKB = T // P           # 16 k-blocks
SCALE = 1.0 / np.sqrt(np.float32(H))  # 1/32
VROWS = 96             # XBAR-transposed V rows (64 d + ones@64 + pad)

# A/B toggles (env: KOPT_<NAME>=0/1)
import os as _os

def _opt(name, default):
    return bool(int(_os.environ.get(f"KOPT_{name}", str(int(default)))))

FP8_PROJ = _opt("FP8PROJ", True)
BODY2 = _opt("BODY2", False)
HEAT = _opt("HEAT", False)


def _build_program(reps: int = 1):
    import contextlib
    import concourse.tile as tile
    from concourse import bacc, mybir
    from concourse.bass import ts

    F32 = mybir.dt.float32
    BF16 = mybir.dt.bfloat16
    F8 = mybir.dt.float8e4
    ActF = mybir.ActivationFunctionType
    DR = mybir.MatmulPerfMode.DoubleRow

    nc = bacc.Bacc("TRN2", target_bir_lowering=False, debug=False)

    xt_d = nc.dram_tensor("xt", [B, P, CB, T], BF16, kind="ExternalInput")
    xt8_d = nc.dram_tensor("xt8", [B, P, CB, T], F8, kind="ExternalInput")
    w_d = {
        n: nc.dram_tensor(f"w{n}t", [H, P], BF16, kind="ExternalInput")
        for n in "qkv"
    }
    w8_d = {
        n: nc.dram_tensor(f"w{n}8", [H, P], F8, kind="ExternalInput")
        for n in "qk"
    }
    b_d = {
        n: nc.dram_tensor(f"b{n}", [P], F32, kind="ExternalInput")
        for n in "qkv"
    }
    out_d = nc.dram_tensor("out", [B, 2, HD + 1, T], F32,
                           kind="ExternalOutput")

    with tile.TileContext(nc) as tc:
        with (
            tc.tile_pool(name="const", bufs=1) as const,
            tc.tile_pool(name="xt", bufs=2) as xt_pool,
            tc.tile_pool(name="vp", bufs=2) as vp_pool,
            tc.tile_pool(name="pt", bufs=4) as pt_pool,
            tc.tile_pool(name="osb", bufs=2) as osb_pool,
            tc.tile_pool(name="psproj", bufs=2, space="PSUM") as psproj,
            tc.tile_pool(name="psS", bufs=2, space="PSUM") as psS_pool,
            tc.tile_pool(name="pso", bufs=1, space="PSUM") as pso,
        ):
            # ---- persistent tiles (fp8 QK path loads first: the
            # prologue projections consume it before anything else) ----
            w8_sb = {}
            for n in "qk":
                w8_sb[n] = const.tile([P, CB, P], F8, tag=f"w8{n}",
                                      name=f"w8{n}")
                nc.sync.dma_start(
                    w8_sb[n][:],
                    w8_d[n][:].rearrange("(cb p) m -> p cb m", p=P),
                )
            w_sb = {}
            bias_sb = {}
            for n in "qkv":
                w_sb[n] = const.tile([P, CB, P], BF16, tag=f"w{n}", name=f"w{n}")
                nc.sync.dma_start(
                    w_sb[n][:],
                    w_d[n][:].rearrange("(cb p) m -> p cb m", p=P),
                )
                bias_sb[n] = const.tile([P, 1], F32, tag=f"b{n}", name=f"b{n}")
                nc.sync.dma_start(bias_sb[n][:], b_d[n][:, None])
            w8_sb = {}
            for n in "qk":
                w8_sb[n] = const.tile([P, CB, P], F8, tag=f"w8{n}",
                                      name=f"w8{n}")
                nc.sync.dma_start(
                    w8_sb[n][:],
                    w8_d[n][:].rearrange("(cb p) m -> p cb m", p=P),
                )

            # fp8 Q/K tiles, [128(2h*64d), 2, T]; [:, 1, :] stays zero
            # (DoubleRow zero-pad contraction tile)
            q8 = {}
            k8 = {}
            for b in range(B):
                q8[b] = const.tile([P, 2, T], F8, tag=f"q8_{b}", name=f"q8_{b}")
                k8[b] = const.tile([P, 2, T], F8, tag=f"k8_{b}", name=f"k8_{b}")
                nc.gpsimd.memset(q8[b][:, 1, :], 0.0)
                nc.gpsimd.memset(k8[b][:, 1, :], 0.0)

            # bf16 V^T staging per head: rows 0:64 = d, row 64 = ones,
            # rows 65:96 = zeros (never read past col 64 after transpose)
            vt = {}
            for b in range(B):
                for h in range(2):
                    t_ = const.tile([VROWS, T], BF16, tag=f"vt{h}_{b}", name=f"vt{h}_{b}")
                    nc.gpsimd.memset(t_[HD:VROWS, :], 0.0)
                    nc.vector.memset(t_[HD : HD + 1, :], 1.0)
                    vt[(h, b)] = t_

            xt_tiles = {}
            vp_tiles = {}

            def load_xt(b):
                # fp8 first (Q/K projections consume it first), chunked
                # by cb pairs so projections pipeline with the transfer;
                # fp8 on the SP DGE queue, bf16 on the ACT DGE queue.
                xtile = xt_pool.tile([P, CB, T], BF16, tag="xt")
                x8tile = xt_pool.tile([P, CB, T], F8, tag="xt8", name="x8tile")
                nc.sync.dma_start(x8tile[:], xt8_d[b])
                for c in range(0, CB, 4):
                    nc.scalar.dma_start(xtile[:, c : c + 4, :],
                                        xt_d[b, :, c : c + 4, :])
                xt_tiles[b] = (xtile, x8tile)

            def proj_slice(b, tt):
                """QKV projections for T-slice tt of batch b, yielded in
                small PE chunks so attention can interleave between them."""
                xtile, x8tile = xt_tiles[b]
                sl = slice(tt * QTILE, (tt + 1) * QTILE)
                for n in "qkv":
                    ps = psproj.tile([P, QTILE], F32, tag="mm")
                    if n == "v" or not FP8_PROJ:
                        for cb in range(CB):
                            nc.tensor.matmul(
                                ps[:],
                                w_sb[n][:, cb, :],
                                xtile[:, cb, sl],
                                start=(cb == 0),
                                stop=(cb == CB - 1),
                            )
                            if cb % 4 == 3:
                                yield
                    else:
                        for c in range(CB // 2):
                            nc.tensor.matmul(
                                ps[:],
                                w8_sb[n][:, 2 * c : 2 * c + 2, :],
                                x8tile[:, 2 * c : 2 * c + 2, sl],
                                start=(c == 0),
                                stop=(c == CB // 2 - 1),
                                perf_mode=DR,
                            )
                            if c % 2 == 1:
                                yield
                    if n == "v":
                        nc.vector.tensor_scalar_add(
                            vt[(0, b)][:HD, sl], ps[:HD, :], bias_sb[n][:HD]
                        )
                        nc.vector.tensor_scalar_add(
                            vt[(1, b)][:HD, sl], ps[HD:, :], bias_sb[n][HD:]
                        )
                    else:
                        dst = q8[b] if n == "q" else k8[b]
                        nc.vector.tensor_scalar_add(
                            dst[:, 0, sl], ps[:], bias_sb[n][:]
                        )
                    yield

            def window_proj(b):
                """Generator: all 4 proj slices + V' transposes for batch
                b, in interleavable chunks."""
                for tt in range(NQT):
                    for _ in proj_slice(b, tt):
                        yield
                    if tt % 2 == 1:
                        vprime_build(b, tt // 2)

            def vprime_alloc(b):
                for h in range(2):
                    vp_tiles[(h, b)] = vp_pool.tile(
                        [P, NKB, VROWS], BF16, tag=f"vp{h}", name=f"vp{h}"
                    )

            def vprime_build(b, half):
                kbs = slice(8 * half, 8 * half + 8)
                sl = slice(half * T // 2, (half + 1) * T // 2)
                for h in range(2):
                    nc.scalar.dma_start_transpose(
                        vp_tiles[(h, b)][:, kbs, :], vt[(h, b)][:, sl]
                    )

            def emit_dummy(b):
                # keeps the PE pipeline hot (DVFS) when no real filler
                # remains: recompute a projection block into a scratch
                # psum tile that nothing reads
                psd = psproj.tile([P, QTILE], F32, tag="mm", name="psd")
                nc.tensor.matmul(
                    psd[:, :QTILE // 2],
                    w8_sb["q"][:, 0:2, :],
                    q8[b][:, :, : QTILE // 2],
                    perf_mode=DR,
                )
                nc.tensor.matmul(
                    psd[:, QTILE // 2 :],
                    w8_sb["k"][:, 0:2, :],
                    k8[b][:, :, : QTILE // 2],
                    perf_mode=DR,
                )

            def attn_qt(b, qt, oSB, filler=None):
                nkb = 4 * qt + 4
                psO = [
                    pso.tile([HD + 1, QTILE], F32, tag=f"o{h}",
                             name=f"psO{h}")
                    for h in range(2)
                ]
                q0 = qt * QTILE
                # diagonal blocks first: their exp+mask chain latency is
                # covered by the remaining full blocks' S matmuls
                kb_order = list(range(4 * qt, nkb)) + list(range(4 * qt))
                first_kb, last_kb = kb_order[0], kb_order[-1]
                pending = None  # (kb, pt, lo) with AV not yet emitted

                def emit_av(kb, pt, lo):
                    for h in range(2):
                        nc.tensor.matmul(
                            psO[h][:, lo:],
                            vp_tiles[(h, b)][:, kb, : HD + 1],
                            pt[:, h, lo:],
                            start=(kb == first_kb),
                            stop=(kb == last_kb),
                        )

                for kb in kb_order:
                    i = kb - 4 * qt
                    lo = max(i, 0) * P
                    psS = psS_pool.tile([P, 2, QTILE], F32, tag="s")
                    for h in range(2):
                        nc.tensor.matmul(
                            psS[:, h, lo:],
                            k8[b][ts(h, HD), :, ts(kb, P)],
                            q8[b][ts(h, HD), :, q0 + lo : q0 + QTILE],
                            perf_mode=DR,
                        )
                    pt = pt_pool.tile([P, 2, QTILE], BF16, tag="pt")
                    nc.scalar.activation(
                        pt[:, :, lo:],
                        psS[:, :, lo:],
                        ActF.Exp,
                        scale=float(SCALE),
                    )
                    if i >= 0:
                        # zero masked (k_local > q_local) on the diagonal
                        # 128-col block, both heads at once
                        nc.gpsimd.affine_select(
                            out=pt[:, :, lo : lo + P],
                            in_=pt[:, :, lo : lo + P],
                            compare_op=mybir.AluOpType.is_ge,
                            fill=0.0,
                            base=0,
                            pattern=[[0, 2], [1, P]],
                            channel_multiplier=-1,
                        )
                    # software pipeline: S(kb) emitted before AV(kb-1) so
                    # the in-order PE never stalls on exp(kb-1)
                    if pending is not None:
                        emit_av(*pending)
                    pending = (kb, pt, lo)
                    if filler is not None:
                        if next(filler, "END") == "END" and HEAT:
                            emit_dummy(b)
                emit_av(*pending)
                for h in range(2):
                    nc.vector.tensor_copy(
                        oSB[h][:, ts(qt, QTILE)], psO[h][:]
                    )

            # ---- prologue: batch 0 projections ----
            load_xt(0)
            vprime_alloc(0)
            for _ in window_proj(0):
                pass

            rep_ctx = (
                tc.For_i(0, reps, 1,
                         hint_engines=(mybir.EngineType.PE,
                                       mybir.EngineType.Activation,
                                       mybir.EngineType.DVE,
                                       mybir.EngineType.Pool,
                                       mybir.EngineType.SP))
                if reps > 1 else contextlib.nullcontext()
            )
            with rep_ctx:
              for _body_i in range(2 if BODY2 else 1):
                load_xt(1)
                vprime_alloc(1)
                oSB = [osb_pool.tile([HD + 1, T], F32, tag=f"oSB{h}",
                                     name=f"oSB{h}") for h in range(2)]
                filler = window_proj(1)
                for qt in range(NQT):
                    attn_qt(0, qt, oSB, filler)
                    if qt == 2:
                        nc.sync.dma_start(out_d[0, 0, :, : 3 * QTILE],
                                          oSB[0][:, : 3 * QTILE])
                        nc.scalar.dma_start(out_d[0, 1, :, : 3 * QTILE],
                                            oSB[1][:, : 3 * QTILE])
                for _ in filler:
                    pass
                load_xt(0)
                nc.sync.dma_start(out_d[0, 0, :, 3 * QTILE :],
                                  oSB[0][:, 3 * QTILE :])
                nc.scalar.dma_start(out_d[0, 1, :, 3 * QTILE :],
                                    oSB[1][:, 3 * QTILE :])
                vprime_alloc(0)
                oSB = [osb_pool.tile([HD + 1, T], F32, tag=f"oSB{h}",
                                     name=f"oSB{h}") for h in range(2)]
                filler = window_proj(0)
                for qt in range(NQT):
                    attn_qt(1, qt, oSB, filler)
                    if qt == 2:
                        nc.sync.dma_start(out_d[1, 0, :, : 3 * QTILE],
                                          oSB[0][:, : 3 * QTILE])
                        nc.scalar.dma_start(out_d[1, 1, :, : 3 * QTILE],
                                            oSB[1][:, : 3 * QTILE])
                for _ in filler:
                    pass
                nc.sync.dma_start(out_d[1, 0, :, 3 * QTILE :],
                                  oSB[0][:, 3 * QTILE :])
                nc.scalar.dma_start(out_d[1, 1, :, 3 * QTILE :],
                                    oSB[1][:, 3 * QTILE :])

    nc.compile()
    return nc


def _make_in_maps(inputs):
    x = np.ascontiguousarray(np.asarray(inputs["x"], np.float32))
    xt_f = np.ascontiguousarray(
        x.transpose(0, 2, 1).reshape(B, CB, P, T).transpose(0, 2, 1, 3)
    )
    xt = xt_f.astype(ml_dtypes.bfloat16)
    xt8 = xt_f.astype(ml_dtypes.float8_e4m3)
    Wq, Wk, Wv = inputs["Wq"], inputs["Wk"], inputs["Wv"]
    bq, bk, bv = inputs["bq"], inputs["bk"], inputs["bv"]

    in_maps = []
    for m in range(NCORES):
        sl = slice(m * P, (m + 1) * P)  # 128 output channels = 2 heads
        in_maps.append({
            "xt": xt,
            "xt8": xt8,
            "wq8": np.ascontiguousarray(
                np.asarray(Wq, np.float32)[sl, :].T
            ).astype(ml_dtypes.float8_e4m3),
            "wk8": np.ascontiguousarray(
                np.asarray(Wk, np.float32)[sl, :].T
            ).astype(ml_dtypes.float8_e4m3),
            "wqt": np.ascontiguousarray(
                np.asarray(Wq, np.float32)[sl, :].T
            ).astype(ml_dtypes.bfloat16),
            "wkt": np.ascontiguousarray(
                np.asarray(Wk, np.float32)[sl, :].T
            ).astype(ml_dtypes.bfloat16),
            "wvt": np.ascontiguousarray(
                np.asarray(Wv, np.float32)[sl, :].T
            ).astype(ml_dtypes.bfloat16),
            "bq": np.ascontiguousarray(np.asarray(bq, np.float32)[sl]),
            "bk": np.ascontiguousarray(np.asarray(bk, np.float32)[sl]),
            "bv": np.ascontiguousarray(np.asarray(bv, np.float32)[sl]),
        })
    return in_maps


_CACHED = {}


def kernel(x, Wq, bq, Wk, bk, Wv, bv):
    from concourse.bass_utils import run_bass_kernel_spmd

    if "nc" not in _CACHED:
        _CACHED["nc"] = _build_program()
    nc = _CACHED["nc"]

    in_maps = _make_in_maps(dict(
        x=x, Wq=Wq, bq=bq, Wk=Wk, bk=bk, Wv=Wv, bv=bv,
    ))

    res = run_bass_kernel_spmd(nc, in_maps, core_ids=list(range(NCORES)))

    parts = []
    for m in range(NCORES):
        o = res.results[m]["out"]  # [B, 2, 65, T] f32
        num = o[:, :, :HD, :]
        z = o[:, :, HD : HD + 1, :]
        core_out = (num / z).transpose(0, 3, 1, 2).reshape(B, T, 2 * HD)
        parts.append(core_out)
    return np.ascontiguousarray(np.concatenate(parts, axis=-1))


# revision 27
# speedup vs baseline: 1.4291x; 1.4291x over previous
"""Multi-head masked self-attention on 8 trn2 NeuronCores.

Problem: B=2, T=2048, H=1024, nH=16 heads (head_dim=64), causal softmax
attention with QKV projections; scores scaled by 1/sqrt(H).

Sharding: heads across cores (2 heads per core), both batches on every
core. QKV weights column-sharded by head: core m gets rows
[128m, 128m+128) of each projection matrix.

Per-core device program (mixed precision, tolerance 2e-2):

  x^T [1024, 2048] streamed in bf16 -> SBUF [128, 8cb, 2048]
  Q^T/K^T [128(2h*64d), T] = Wn^T @ x^T in bf16 (PE, 8-block f32 PSUM
      accumulation); DVE evicts with per-partition bias add straight to
      fp8(e4m3) tiles laid out [128, 2, T] whose second contraction tile
      is persistent zeros (DoubleRow zero-padding).
  V^T likewise but evicted to bf16 per-head tiles vt_h [96, T] (head1
      via cross-partition DVE copy 64:128 -> 0:64); row 64 = ones.
      One XBAR dma_start_transpose per head builds V' [128k, 16kb, 96]
      with column 64 = ones (softmax denominator column).
  Scores (transposed): S^T[k, q] fp8 DoubleRow matmuls (0.5 cyc/row),
      contraction [64, 2, .] zero-padded.
      ACT evicts exp(S/32) -> bf16 P tiles [128, 2, 512]; causal mask
      applied post-exp as gpsimd affine_select zeroing on the diagonal
      128-col block (both heads in one op).
  O'^T [65, 512] = sum_kb V'[:, kb, :65].T @ P^T (bf16 PE accumulation;
      row 64 accumulates Z). DVE evicts psO -> SBUF f32, DMA to DRAM
      out [B, 2, 65, T]. Host divides by Z and transposes (cheap).

  Emission is software-pipelined at two levels: (1) within a q-tile,
  S(kb+1) is emitted before AV(kb) so the in-order PE never stalls on
  exp(kb); (2) across batches, b1's projection chunks are interleaved
  per-kb into b0's attention (attention is ACT/exp-bound, projections
  are PE-bound), and b0's projections for the next rep overlap b1's
  attention. All input DMAs are issued on the SP DGE queue and outputs
  on the GpSimd SWDGE queue so the Activation sequencer dispatches
  nothing but exp (DMA issue on the ACT queue was measured to cost
  ~15us/rep of exp stalls).
"""
import sys

sys.path.insert(0, "/opt/trn_rl_repo")

import numpy as np
import ml_dtypes

B = 2
T = 2048
H = 1024
NHEADS = 16
HD = 64
NCORES = 8
P = 128
CB = H // P            # 8 contraction blocks for projections
QTILE = 512
NQT = T // QTILE       # 4 q-tiles
NKB = T // P           # 16 k-blocks
SCALE = 1.0 / np.sqrt(np.float32(H))  # 1/32
VROWS = 96             # XBAR-transposed V rows (64 d + ones@64 + pad)

# A/B toggles (env: KOPT_<NAME>=0/1)
import os as _os

def _opt(name, default):
    return bool(int(_os.environ.get(f"KOPT_{name}", str(int(default)))))


def _opt2(name, default):
    return int(_os.environ.get(f"KOPT_{name}", str(default)))

_DEFAULT_OPTS = dict(
    FP8_PROJ=_opt("FP8PROJ", True),
    BODY2=_opt("BODY2", False),
    HEAT=_opt("HEAT", False),
    X8GP=_opt("X8GP", False),
    MASKDVE=_opt("MASKDVE", False),
    PTBUFS=_opt2("PTBUFS", 4),
    SCHUNK=_opt2("SCHUNK", 512),
)


def _build_program(reps: int = 1, **overrides):
    opts = dict(_DEFAULT_OPTS, **overrides)
    FP8_PROJ = opts["FP8_PROJ"]
    BODY2 = opts["BODY2"]
    HEAT = opts["HEAT"]
    X8GP = opts["X8GP"]
    MASKDVE = opts["MASKDVE"]
    PTBUFS = opts["PTBUFS"]
    SCHUNK = opts["SCHUNK"]
    import contextlib
    import concourse.tile as tile
    from concourse import bacc, mybir
    from concourse.bass import ts

    F32 = mybir.dt.float32
    BF16 = mybir.dt.bfloat16
    F8 = mybir.dt.float8e4
    ActF = mybir.ActivationFunctionType
    DR = mybir.MatmulPerfMode.DoubleRow

    nc = bacc.Bacc("TRN2", target_bir_lowering=False, debug=False)

    xt_d = nc.dram_tensor("xt", [B, P, CB, T], BF16, kind="ExternalInput")
    xt8_d = nc.dram_tensor("xt8", [B, P, CB, T], F8, kind="ExternalInput")
    w_d = {
        n: nc.dram_tensor(f"w{n}t", [H, P], BF16, kind="ExternalInput")
        for n in "qkv"
    }
    w8_d = {
        n: nc.dram_tensor(f"w{n}8", [H, P], F8, kind="ExternalInput")
        for n in "qk"
    }
    b_d = {
        n: nc.dram_tensor(f"b{n}", [P], F32, kind="ExternalInput")
        for n in "qkv"
    }
    out_d = nc.dram_tensor("out", [B, 2, HD + 1, T], F32,
                           kind="ExternalOutput")

    with tile.TileContext(nc) as tc:
        with (
            tc.tile_pool(name="const", bufs=1) as const,
            tc.tile_pool(name="xt", bufs=2) as xt_pool,
            tc.tile_pool(name="vp", bufs=2) as vp_pool,
            tc.tile_pool(name="pt", bufs=PTBUFS) as pt_pool,
            tc.tile_pool(name="osb", bufs=2) as osb_pool,
            tc.tile_pool(name="psproj", bufs=2, space="PSUM") as psproj,
            tc.tile_pool(name="psS", bufs=2, space="PSUM") as psS_pool,
            tc.tile_pool(name="pso", bufs=1, space="PSUM") as pso,
        ):
            # ---- persistent tiles (fp8 QK path loads first: the
            # prologue projections consume it before anything else) ----
            w8_sb = {}
            for n in "qk":
                w8_sb[n] = const.tile([P, CB, P], F8, tag=f"w8{n}",
                                      name=f"w8{n}")
                nc.sync.dma_start(
                    w8_sb[n][:],
                    w8_d[n][:].rearrange("(cb p) m -> p cb m", p=P),
                )
            w_sb = {}
            bias_sb = {}
            for n in "qkv":
                w_sb[n] = const.tile([P, CB, P], BF16, tag=f"w{n}", name=f"w{n}")
                nc.sync.dma_start(
                    w_sb[n][:],
                    w_d[n][:].rearrange("(cb p) m -> p cb m", p=P),
                )
                bias_sb[n] = const.tile([P, 1], F32, tag=f"b{n}", name=f"b{n}")
                nc.sync.dma_start(bias_sb[n][:], b_d[n][:, None])
            w8_sb = {}
            for n in "qk":
                w8_sb[n] = const.tile([P, CB, P], F8, tag=f"w8{n}",
                                      name=f"w8{n}")
                nc.sync.dma_start(
                    w8_sb[n][:],
                    w8_d[n][:].rearrange("(cb p) m -> p cb m", p=P),
                )

            # fp8 Q/K tiles, [128(2h*64d), 2, T]; [:, 1, :] stays zero
            # (DoubleRow zero-pad contraction tile)
            q8 = {}
            k8 = {}
            for b in range(B):
                q8[b] = const.tile([P, 2, T], F8, tag=f"q8_{b}", name=f"q8_{b}")
                k8[b] = const.tile([P, 2, T], F8, tag=f"k8_{b}", name=f"k8_{b}")
                nc.gpsimd.memset(q8[b][:, 1, :], 0.0)
                nc.gpsimd.memset(k8[b][:, 1, :], 0.0)

            # bf16 V^T staging per head: rows 0:64 = d, row 64 = ones,
            # rows 65:96 = zeros (never read past col 64 after transpose)
            vt = {}
            for b in range(B):
                for h in range(2):
                    t_ = const.tile([VROWS, T], BF16, tag=f"vt{h}_{b}", name=f"vt{h}_{b}")
                    nc.gpsimd.memset(t_[HD:VROWS, :], 0.0)
                    nc.vector.memset(t_[HD : HD + 1, :], 1.0)
                    vt[(h, b)] = t_

            # {0,1} bf16 causal mask for the diagonal 128-col block,
            # replicated for both heads (DVE multiply masks post-exp P)
            trimask = const.tile([P, 2, P], BF16, tag="trimask")
            nc.vector.memset(trimask[:], 1.0)
            nc.gpsimd.affine_select(
                out=trimask[:],
                in_=trimask[:],
                compare_op=mybir.AluOpType.is_ge,
                fill=0.0,
                base=0,
                pattern=[[0, 2], [1, P]],
                channel_multiplier=-1,
            )

            xt_tiles = {}
            vp_tiles = {}

            def load_xt(b, act_queue=False):
                # fp8 first (Q/K projections consume it first). Both on
                # the SP DGE queue in the steady state -- the ACT queue
                # must stay clear so the Activation sequencer only
                # dispatches exps. The prologue instance may use the ACT
                # queue (no exps yet) to parallelize the initial load.
                xtile = xt_pool.tile([P, CB, T], BF16, tag="xt")
                x8tile = xt_pool.tile([P, CB, T], F8, tag="xt8", name="x8tile")
                if X8GP:
                    nc.gpsimd.dma_start(x8tile[:], xt8_d[b])
                else:
                    nc.sync.dma_start(x8tile[:], xt8_d[b])
                if act_queue:
                    nc.scalar.dma_start(xtile[:], xt_d[b])
                else:
                    nc.sync.dma_start(xtile[:], xt_d[b])
                xt_tiles[b] = (xtile, x8tile)

            def proj_slice(b, tt):
                """QKV projections for T-slice tt of batch b, yielded in
                small PE chunks so attention can interleave between them."""
                xtile, x8tile = xt_tiles[b]
                sl = slice(tt * QTILE, (tt + 1) * QTILE)
                for n in "qkv":
                    ps = psproj.tile([P, QTILE], F32, tag="mm")
                    if n == "v" or not FP8_PROJ:
                        for cb in range(CB):
                            nc.tensor.matmul(
                                ps[:],
                                w_sb[n][:, cb, :],
                                xtile[:, cb, sl],
                                start=(cb == 0),
                                stop=(cb == CB - 1),
                            )
                            if cb % 4 == 3:
                                yield
                    else:
                        for c in range(CB // 2):
                            nc.tensor.matmul(
                                ps[:],
                                w8_sb[n][:, 2 * c : 2 * c + 2, :],
                                x8tile[:, 2 * c : 2 * c + 2, sl],
                                start=(c == 0),
                                stop=(c == CB // 2 - 1),
                                perf_mode=DR,
                            )
                            if c % 2 == 1:
                                yield
                    if n == "v":
                        nc.vector.tensor_scalar_add(
                            vt[(0, b)][:HD, sl], ps[:HD, :], bias_sb[n][:HD]
                        )
                        nc.vector.tensor_scalar_add(
                            vt[(1, b)][:HD, sl], ps[HD:, :], bias_sb[n][HD:]
                        )
                    else:
                        dst = q8[b] if n == "q" else k8[b]
                        nc.vector.tensor_scalar_add(
                            dst[:, 0, sl], ps[:], bias_sb[n][:]
                        )
                    yield

            def window_proj(b):
                """Generator: all 4 proj slices + V' transposes for batch
                b, in interleavable chunks."""
                for tt in range(NQT):
                    for _ in proj_slice(b, tt):
                        yield
                    if tt % 2 == 1:
                        vprime_build(b, tt // 2)

            def vprime_alloc(b):
                for h in range(2):
                    vp_tiles[(h, b)] = vp_pool.tile(
                        [P, NKB, VROWS], BF16, tag=f"vp{h}", name=f"vp{h}"
                    )

            def vprime_build(b, half):
                kbs = slice(8 * half, 8 * half + 8)
                sl = slice(half * T // 2, (half + 1) * T // 2)
                for h in range(2):
                    nc.sync.dma_start_transpose(
                        vp_tiles[(h, b)][:, kbs, :], vt[(h, b)][:, sl]
                    )

            def emit_dummy(b):
                # keeps the PE pipeline hot (DVFS) when no real filler
                # remains: recompute a projection block into a scratch
                # psum tile that nothing reads
                psd = psproj.tile([P, QTILE], F32, tag="mm", name="psd")
                nc.tensor.matmul(
                    psd[:, :QTILE // 2],
                    w8_sb["q"][:, 0:2, :],
                    q8[b][:, :, : QTILE // 2],
                    perf_mode=DR,
                )
                nc.tensor.matmul(
                    psd[:, QTILE // 2 :],
                    w8_sb["k"][:, 0:2, :],
                    k8[b][:, :, : QTILE // 2],
                    perf_mode=DR,
                )

            def attn_qt(b, qt, oSB, filler=None):
                nkb = 4 * qt + 4
                psO = [
                    pso.tile([HD + 1, QTILE], F32, tag=f"o{h}",
                             name=f"psO{h}")
                    for h in range(2)
                ]
                q0 = qt * QTILE
                kb_order = list(range(nkb))
                first_kb, last_kb = kb_order[0], kb_order[-1]
                pending = None  # (kb, pt, lo) with AV not yet emitted

                def emit_av(kb, pt, lo):
                    for h in range(2):
                        nc.tensor.matmul(
                            psO[h][:, lo:],
                            vp_tiles[(h, b)][:, kb, : HD + 1],
                            pt[:, h, lo:],
                            start=(kb == first_kb),
                            stop=(kb == last_kb),
                        )

                for kb in kb_order:
                    i = kb - 4 * qt
                    lo = max(i, 0) * P
                    psS = psS_pool.tile([P, 2, QTILE], F32, tag="s")
                    for h in range(2):
                        c0 = lo
                        while c0 < QTILE:
                            n_ = min(SCHUNK, QTILE - c0)
                            nc.tensor.matmul(
                                psS[:, h, c0 : c0 + n_],
                                k8[b][ts(h, HD), :, ts(kb, P)],
                                q8[b][ts(h, HD), :,
                                      q0 + c0 : q0 + c0 + n_],
                                perf_mode=DR,
                            )
                            c0 += n_
                    pt = pt_pool.tile([P, 2, QTILE], BF16, tag="pt")
                    nc.scalar.activation(
                        pt[:, :, lo:],
                        psS[:, :, lo:],
                        ActF.Exp,
                        scale=float(SCALE),
                    )
                    if i >= 0:
                        # zero masked (k_local > q_local) on the diagonal
                        # 128-col block, both heads at once
                        if MASKDVE:
                            nc.vector.tensor_mul(
                                pt[:, :, lo : lo + P],
                                pt[:, :, lo : lo + P],
                                trimask[:],
                            )
                        else:
                            nc.gpsimd.affine_select(
                                out=pt[:, :, lo : lo + P],
                                in_=pt[:, :, lo : lo + P],
                                compare_op=mybir.AluOpType.is_ge,
                                fill=0.0,
                                base=0,
                                pattern=[[0, 2], [1, P]],
                                channel_multiplier=-1,
                            )
                    # software pipeline: S(kb) emitted before AV(kb-1) so
                    # the in-order PE never stalls on exp(kb-1)
                    if pending is not None:
                        emit_av(*pending)
                    pending = (kb, pt, lo)
                    if filler is not None:
                        if next(filler, "END") == "END" and HEAT:
                            emit_dummy(b)
                emit_av(*pending)
                for h in range(2):
                    nc.vector.tensor_copy(
                        oSB[h][:, ts(qt, QTILE)], psO[h][:]
                    )

            # ---- prologue: batch 0 projections ----
            load_xt(0)
            vprime_alloc(0)
            for _ in window_proj(0):
                pass

            rep_ctx = (
                tc.For_i(0, reps, 1,
                         hint_engines=(mybir.EngineType.PE,
                                       mybir.EngineType.Activation,
                                       mybir.EngineType.DVE,
                                       mybir.EngineType.Pool,
                                       mybir.EngineType.SP))
                if reps > 1 else contextlib.nullcontext()
            )
            with rep_ctx:
              for _body_i in range(2 if BODY2 else 1):
                load_xt(1)
                vprime_alloc(1)
                oSB = [osb_pool.tile([HD + 1, T], F32, tag=f"oSB{h}",
                                     name=f"oSB{h}") for h in range(2)]
                filler = window_proj(1)
                for qt in range(NQT):
                    attn_qt(0, qt, oSB, filler)
                    if qt == 2:
                        nc.gpsimd.dma_start(out_d[0, 0, :, : 3 * QTILE],
                                            oSB[0][:, : 3 * QTILE])
                        nc.gpsimd.dma_start(out_d[0, 1, :, : 3 * QTILE],
                                            oSB[1][:, : 3 * QTILE])
                for _ in filler:
                    pass
                load_xt(0)
                nc.gpsimd.dma_start(out_d[0, 0, :, 3 * QTILE :],
                                    oSB[0][:, 3 * QTILE :])
                nc.gpsimd.dma_start(out_d[0, 1, :, 3 * QTILE :],
                                    oSB[1][:, 3 * QTILE :])
                vprime_alloc(0)
                oSB = [osb_pool.tile([HD + 1, T], F32, tag=f"oSB{h}",
                                     name=f"oSB{h}") for h in range(2)]
                filler = window_proj(0)
                for qt in range(NQT):
                    attn_qt(1, qt, oSB, filler)
                    if qt == 2:
                        nc.gpsimd.dma_start(out_d[1, 0, :, : 3 * QTILE],
                                            oSB[0][:, : 3 * QTILE])
                        nc.gpsimd.dma_start(out_d[1, 1, :, : 3 * QTILE],
                                            oSB[1][:, : 3 * QTILE])
                for _ in filler:
                    pass
                nc.gpsimd.dma_start(out_d[1, 0, :, 3 * QTILE :],
                                    oSB[0][:, 3 * QTILE :])
                nc.gpsimd.dma_start(out_d[1, 1, :, 3 * QTILE :],
                                    oSB[1][:, 3 * QTILE :])

    nc.compile()
    return nc


def _make_in_maps(inputs):
    x = np.ascontiguousarray(np.asarray(inputs["x"], np.float32))
    xt_f = np.ascontiguousarray(
        x.transpose(0, 2, 1).reshape(B, CB, P, T).transpose(0, 2, 1, 3)
    )
    xt = xt_f.astype(ml_dtypes.bfloat16)
    xt8 = xt_f.astype(ml_dtypes.float8_e4m3)
    Wq, Wk, Wv = inputs["Wq"], inputs["Wk"], inputs["Wv"]
    bq, bk, bv = inputs["bq"], inputs["bk"], inputs["bv"]

    in_maps = []
    for m in range(NCORES):
        sl = slice(m * P, (m + 1) * P)  # 128 output channels = 2 heads
        in_maps.append({
            "xt": xt,
            "xt8": xt8,
            "wq8": np.ascontiguousarray(
                np.asarray(Wq, np.float32)[sl, :].T
            ).astype(ml_dtypes.float8_e4m3),
            "wk8": np.ascontiguousarray(
                np.asarray(Wk, np.float32)[sl, :].T
            ).astype(ml_dtypes.float8_e4m3),
            "wqt": np.ascontiguousarray(
                np.asarray(Wq, np.float32)[sl, :].T
            ).astype(ml_dtypes.bfloat16),
            "wkt": np.ascontiguousarray(
                np.asarray(Wk, np.float32)[sl, :].T
            ).astype(ml_dtypes.bfloat16),
            "wvt": np.ascontiguousarray(
                np.asarray(Wv, np.float32)[sl, :].T
            ).astype(ml_dtypes.bfloat16),
            "bq": np.ascontiguousarray(np.asarray(bq, np.float32)[sl]),
            "bk": np.ascontiguousarray(np.asarray(bk, np.float32)[sl]),
            "bv": np.ascontiguousarray(np.asarray(bv, np.float32)[sl]),
        })
    return in_maps


_CACHED = {}


def kernel(x, Wq, bq, Wk, bk, Wv, bv):
    from concourse.bass_utils import run_bass_kernel_spmd

    if "nc" not in _CACHED:
        _CACHED["nc"] = _build_program()
    nc = _CACHED["nc"]

    in_maps = _make_in_maps(dict(
        x=x, Wq=Wq, bq=bq, Wk=Wk, bk=bk, Wv=Wv, bv=bv,
    ))

    res = run_bass_kernel_spmd(nc, in_maps, core_ids=list(range(NCORES)))

    parts = []
    for m in range(NCORES):
        o = res.results[m]["out"]  # [B, 2, 65, T] f32
        num = o[:, :, :HD, :]
        z = o[:, :, HD : HD + 1, :]
        core_out = (num / z).transpose(0, 3, 1, 2).reshape(B, T, 2 * HD)
        parts.append(core_out)
    return np.ascontiguousarray(np.concatenate(parts, axis=-1))


# revision 34
# speedup vs baseline: 1.4675x; 1.0269x over previous
"""Multi-head masked self-attention on 8 trn2 NeuronCores.

Problem: B=2, T=2048, H=1024, nH=16 heads (head_dim=64), causal softmax
attention with QKV projections; scores scaled by 1/sqrt(H).

Sharding: heads across cores (2 heads per core), both batches on every
core. QKV weights column-sharded by head: core m gets rows
[128m, 128m+128) of each projection matrix.

Per-core device program (mixed precision, tolerance 2e-2):

  x^T [1024, 2048] streamed in bf16 -> SBUF [128, 8cb, 2048]
  Q^T/K^T [128(2h*64d), T] = Wn^T @ x^T in bf16 (PE, 8-block f32 PSUM
      accumulation); DVE evicts with per-partition bias add straight to
      fp8(e4m3) tiles laid out [128, 2, T] whose second contraction tile
      is persistent zeros (DoubleRow zero-padding).
  V^T likewise but evicted to bf16 per-head tiles vt_h [96, T] (head1
      via cross-partition DVE copy 64:128 -> 0:64); row 64 = ones.
      One XBAR dma_start_transpose per head builds V' [128k, 16kb, 96]
      with column 64 = ones (softmax denominator column).
  Scores (transposed): S^T[k, q] fp8 DoubleRow matmuls (0.5 cyc/row),
      contraction [64, 2, .] zero-padded.
      ACT evicts exp(S/32) -> bf16 P tiles [128, 2, 512]; causal mask
      applied post-exp as gpsimd affine_select zeroing on the diagonal
      128-col block (both heads in one op).
  O'^T [65, 512] = sum_kb V'[:, kb, :65].T @ P^T (bf16 PE accumulation;
      row 64 accumulates Z). DVE evicts psO -> SBUF f32, DMA to DRAM
      out [B, 2, 65, T]. Host divides by Z and transposes (cheap).

  Emission is software-pipelined at two levels: (1) within a q-tile,
  S(kb+1) is emitted before AV(kb) so the in-order PE never stalls on
  exp(kb); (2) across batches, b1's projection chunks are interleaved
  per-kb into b0's attention (attention is ACT/exp-bound, projections
  are PE-bound), and b0's projections for the next rep overlap b1's
  attention. All DMAs (inputs and outputs) are issued on the SP DGE
  queue so the Activation sequencer dispatches nothing but exp (DMA
  issue on the ACT queue was measured to cost ~15us/rep of exp stalls,
  and output stores on the GpSimd SWDGE queue delayed the Pool-engine
  causal-mask ops by ~5us/rep).
"""
import sys

sys.path.insert(0, "/opt/trn_rl_repo")

import numpy as np
import ml_dtypes

B = 2
T = 2048
H = 1024
NHEADS = 16
HD = 64
NCORES = 8
P = 128
CB = H // P            # 8 contraction blocks for projections
QTILE = 512
NQT = T // QTILE       # 4 q-tiles
NKB = T // P           # 16 k-blocks
SCALE = 1.0 / np.sqrt(np.float32(H))  # 1/32
VROWS = 96             # XBAR-transposed V rows (64 d + ones@64 + pad)

# A/B toggles (env: KOPT_<NAME>=0/1)
import os as _os

def _opt(name, default):
    return bool(int(_os.environ.get(f"KOPT_{name}", str(int(default)))))


def _opt2(name, default):
    return int(_os.environ.get(f"KOPT_{name}", str(default)))

_DEFAULT_OPTS = dict(
    FP8_PROJ=_opt("FP8PROJ", True),
    BODY2=_opt("BODY2", False),
    HEAT=_opt("HEAT", False),
    X8GP=_opt("X8GP", False),
    MASKDVE=_opt("MASKDVE", False),
    PTBUFS=_opt2("PTBUFS", 4),
    SCHUNK=_opt2("SCHUNK", 512),
    AVLAG=_opt2("AVLAG", 1),
    OUTSP=_opt("OUTSP", True),
    EXP2=_opt("EXP2", False),
)


def _build_program(reps: int = 1, **overrides):
    opts = dict(_DEFAULT_OPTS, **overrides)
    FP8_PROJ = opts["FP8_PROJ"]
    BODY2 = opts["BODY2"]
    HEAT = opts["HEAT"]
    X8GP = opts["X8GP"]
    MASKDVE = opts["MASKDVE"]
    PTBUFS = opts["PTBUFS"]
    SCHUNK = opts["SCHUNK"]
    AVLAG = opts["AVLAG"]
    OUTSP = opts["OUTSP"]
    EXP2 = opts["EXP2"]
    import contextlib
    import concourse.tile as tile
    from concourse import bacc, mybir
    from concourse.bass import ts

    F32 = mybir.dt.float32
    BF16 = mybir.dt.bfloat16
    F8 = mybir.dt.float8e4
    ActF = mybir.ActivationFunctionType
    DR = mybir.MatmulPerfMode.DoubleRow

    nc = bacc.Bacc("TRN2", target_bir_lowering=False, debug=False)

    xt_d = nc.dram_tensor("xt", [B, P, CB, T], BF16, kind="ExternalInput")
    xt8_d = nc.dram_tensor("xt8", [B, P, CB, T], F8, kind="ExternalInput")
    w_d = {
        n: nc.dram_tensor(f"w{n}t", [H, P], BF16, kind="ExternalInput")
        for n in "qkv"
    }
    w8_d = {
        n: nc.dram_tensor(f"w{n}8", [H, P], F8, kind="ExternalInput")
        for n in "qk"
    }
    b_d = {
        n: nc.dram_tensor(f"b{n}", [P], F32, kind="ExternalInput")
        for n in "qkv"
    }
    out_d = nc.dram_tensor("out", [B, 2, HD + 1, T], F32,
                           kind="ExternalOutput")

    with tile.TileContext(nc) as tc:
        with (
            tc.tile_pool(name="const", bufs=1) as const,
            tc.tile_pool(name="xt", bufs=2) as xt_pool,
            tc.tile_pool(name="vp", bufs=2) as vp_pool,
            tc.tile_pool(name="pt", bufs=PTBUFS) as pt_pool,
            tc.tile_pool(name="osb", bufs=2) as osb_pool,
            tc.tile_pool(name="psproj", bufs=2, space="PSUM") as psproj,
            tc.tile_pool(name="psS", bufs=(1 if EXP2 else 2),
                         space="PSUM") as psS_pool,
            tc.tile_pool(name="pso", bufs=1, space="PSUM") as pso,
        ):
            # ---- persistent tiles (fp8 QK path loads first: the
            # prologue projections consume it before anything else) ----
            w8_sb = {}
            for n in "qk":
                w8_sb[n] = const.tile([P, CB, P], F8, tag=f"w8{n}",
                                      name=f"w8{n}")
                nc.sync.dma_start(
                    w8_sb[n][:],
                    w8_d[n][:].rearrange("(cb p) m -> p cb m", p=P),
                )
            w_sb = {}
            bias_sb = {}
            for n in "qkv":
                w_sb[n] = const.tile([P, CB, P], BF16, tag=f"w{n}", name=f"w{n}")
                nc.sync.dma_start(
                    w_sb[n][:],
                    w_d[n][:].rearrange("(cb p) m -> p cb m", p=P),
                )
                bias_sb[n] = const.tile([P, 1], F32, tag=f"b{n}", name=f"b{n}")
                nc.sync.dma_start(bias_sb[n][:], b_d[n][:, None])

            # fp8 Q/K tiles, [128(2h*64d), 2, T]; [:, 1, :] stays zero
            # (DoubleRow zero-pad contraction tile)
            q8 = {}
            k8 = {}
            for b in range(B):
                q8[b] = const.tile([P, 2, T], F8, tag=f"q8_{b}", name=f"q8_{b}")
                k8[b] = const.tile([P, 2, T], F8, tag=f"k8_{b}", name=f"k8_{b}")
                nc.gpsimd.memset(q8[b][:, 1, :], 0.0)
                nc.gpsimd.memset(k8[b][:, 1, :], 0.0)

            # bf16 V^T staging per head: rows 0:64 = d, row 64 = ones,
            # rows 65:96 = zeros (never read past col 64 after transpose)
            vt = {}
            for b in range(B):
                for h in range(2):
                    t_ = const.tile([VROWS, T], BF16, tag=f"vt{h}_{b}", name=f"vt{h}_{b}")
                    nc.gpsimd.memset(t_[HD:VROWS, :], 0.0)
                    nc.vector.memset(t_[HD : HD + 1, :], 1.0)
                    vt[(h, b)] = t_

            # {0,1} bf16 causal mask for the diagonal 128-col block,
            # replicated for both heads (DVE multiply masks post-exp P)
            trimask = const.tile([P, 2, P], BF16, tag="trimask")
            nc.vector.memset(trimask[:], 1.0)
            nc.gpsimd.affine_select(
                out=trimask[:],
                in_=trimask[:],
                compare_op=mybir.AluOpType.is_ge,
                fill=0.0,
                base=0,
                pattern=[[0, 2], [1, P]],
                channel_multiplier=-1,
            )

            def out_q(dst, srctile):
                if OUTSP:
                    nc.sync.dma_start(dst, srctile)
                else:
                    nc.gpsimd.dma_start(dst, srctile)

            xt_tiles = {}
            vp_tiles = {}

            def load_xt(b, act_queue=False):
                # fp8 first (Q/K projections consume it first). Both on
                # the SP DGE queue in the steady state -- the ACT queue
                # must stay clear so the Activation sequencer only
                # dispatches exps. The prologue instance may use the ACT
                # queue (no exps yet) to parallelize the initial load.
                xtile = xt_pool.tile([P, CB, T], BF16, tag="xt")
                x8tile = xt_pool.tile([P, CB, T], F8, tag="xt8", name="x8tile")
                if X8GP:
                    nc.gpsimd.dma_start(x8tile[:], xt8_d[b])
                else:
                    nc.sync.dma_start(x8tile[:], xt8_d[b])
                if act_queue:
                    nc.scalar.dma_start(xtile[:], xt_d[b])
                else:
                    nc.sync.dma_start(xtile[:], xt_d[b])
                xt_tiles[b] = (xtile, x8tile)

            def proj_slice(b, tt):
                """QKV projections for T-slice tt of batch b, yielded in
                small PE chunks so attention can interleave between them."""
                xtile, x8tile = xt_tiles[b]
                sl = slice(tt * QTILE, (tt + 1) * QTILE)
                for n in "qkv":
                    ps = psproj.tile([P, QTILE], F32, tag="mm")
                    if n == "v" or not FP8_PROJ:
                        for cb in range(CB):
                            nc.tensor.matmul(
                                ps[:],
                                w_sb[n][:, cb, :],
                                xtile[:, cb, sl],
                                start=(cb == 0),
                                stop=(cb == CB - 1),
                            )
                            if cb % 4 == 3:
                                yield
                    else:
                        for c in range(CB // 2):
                            nc.tensor.matmul(
                                ps[:],
                                w8_sb[n][:, 2 * c : 2 * c + 2, :],
                                x8tile[:, 2 * c : 2 * c + 2, sl],
                                start=(c == 0),
                                stop=(c == CB // 2 - 1),
                                perf_mode=DR,
                            )
                            if c % 2 == 1:
                                yield
                    if n == "v":
                        nc.vector.tensor_scalar_add(
                            vt[(0, b)][:HD, sl], ps[:HD, :], bias_sb[n][:HD]
                        )
                        nc.vector.tensor_scalar_add(
                            vt[(1, b)][:HD, sl], ps[HD:, :], bias_sb[n][HD:]
                        )
                    else:
                        dst = q8[b] if n == "q" else k8[b]
                        nc.vector.tensor_scalar_add(
                            dst[:, 0, sl], ps[:], bias_sb[n][:]
                        )
                    yield

            def window_proj(b):
                """Generator: all 4 proj slices + V' transposes for batch
                b, in interleavable chunks."""
                for tt in range(NQT):
                    for _ in proj_slice(b, tt):
                        yield
                    if tt % 2 == 1:
                        vprime_build(b, tt // 2)

            def vprime_alloc(b):
                for h in range(2):
                    vp_tiles[(h, b)] = vp_pool.tile(
                        [P, NKB, VROWS], BF16, tag=f"vp{h}", name=f"vp{h}"
                    )

            def vprime_build(b, half):
                kbs = slice(8 * half, 8 * half + 8)
                sl = slice(half * T // 2, (half + 1) * T // 2)
                for h in range(2):
                    nc.sync.dma_start_transpose(
                        vp_tiles[(h, b)][:, kbs, :], vt[(h, b)][:, sl]
                    )

            def emit_dummy(b):
                # keeps the PE pipeline hot (DVFS) when no real filler
                # remains: recompute a projection block into a scratch
                # psum tile that nothing reads
                psd = psproj.tile([P, QTILE], F32, tag="mm", name="psd")
                nc.tensor.matmul(
                    psd[:, :QTILE // 2],
                    w8_sb["q"][:, 0:2, :],
                    q8[b][:, :, : QTILE // 2],
                    perf_mode=DR,
                )
                nc.tensor.matmul(
                    psd[:, QTILE // 2 :],
                    w8_sb["k"][:, 0:2, :],
                    k8[b][:, :, : QTILE // 2],
                    perf_mode=DR,
                )

            def attn_qt(b, qt, oSB, filler=None):
                nkb = 4 * qt + 4
                psO = [
                    pso.tile([HD + 1, QTILE], F32, tag=f"o{h}",
                             name=f"psO{h}")
                    for h in range(2)
                ]
                q0 = qt * QTILE
                kb_order = list(range(nkb))
                first_kb, last_kb = kb_order[0], kb_order[-1]
                pending = []  # [(kb, pt, lo)] AVs not yet emitted

                def emit_av(kb, pt, lo):
                    for h in range(2):
                        nc.tensor.matmul(
                            psO[h][:, lo:],
                            vp_tiles[(h, b)][:, kb, : HD + 1],
                            pt[:, h, lo:],
                            start=(kb == first_kb),
                            stop=(kb == last_kb),
                        )

                if EXP2:
                    # k-blocks processed in fused pairs: one 4-bank psS
                    # tile and ONE exp per pair (halves the ACT op count
                    # and the S->exp->AV semaphore chains)
                    pairs = [(kb_order[x], kb_order[x + 1])
                             for x in range(0, nkb, 2)]
                    for pair in pairs:
                        psS = psS_pool.tile([P, 2, 2, QTILE], F32, tag="s")
                        pt = pt_pool.tile([P, 2, 2, QTILE], BF16, tag="pt")
                        los = []
                        for j, kb in enumerate(pair):
                            lo = max(kb - 4 * qt, 0) * P
                            los.append(lo)
                            for h in range(2):
                                nc.tensor.matmul(
                                    psS[:, j, h, lo:],
                                    k8[b][ts(h, HD), :, ts(kb, P)],
                                    q8[b][ts(h, HD), :, q0 + lo :
                                          q0 + QTILE],
                                    perf_mode=DR,
                                )
                        lo0 = los[0]
                        nc.scalar.activation(
                            pt[:, :, :, lo0:],
                            psS[:, :, :, lo0:],
                            ActF.Exp,
                            scale=float(SCALE),
                        )
                        for j, kb in enumerate(pair):
                            if kb - 4 * qt >= 0:
                                lo = los[j]
                                nc.gpsimd.affine_select(
                                    out=pt[:, j, :, lo : lo + P],
                                    in_=pt[:, j, :, lo : lo + P],
                                    compare_op=mybir.AluOpType.is_ge,
                                    fill=0.0,
                                    base=0,
                                    pattern=[[0, 2], [1, P]],
                                    channel_multiplier=-1,
                                )
                        pending.append((pair, pt, los))
                        if len(pending) > 1:
                            ppair, ppt, plos = pending.pop(0)
                            for j, kb in enumerate(ppair):
                                emit_av(kb, ppt[:, j], plos[j])
                        if filler is not None:
                            for _ in range(2):
                                if next(filler, "END") == "END" and HEAT:
                                    emit_dummy(b)
                    while pending:
                        ppair, ppt, plos = pending.pop(0)
                        for j, kb in enumerate(ppair):
                            emit_av(kb, ppt[:, j], plos[j])
                else:
                  for kb in kb_order:
                    i = kb - 4 * qt
                    lo = max(i, 0) * P
                    psS = psS_pool.tile([P, 2, QTILE], F32, tag="s")
                    for h in range(2):
                        c0 = lo
                        while c0 < QTILE:
                            n_ = min(SCHUNK, QTILE - c0)
                            nc.tensor.matmul(
                                psS[:, h, c0 : c0 + n_],
                                k8[b][ts(h, HD), :, ts(kb, P)],
                                q8[b][ts(h, HD), :,
                                      q0 + c0 : q0 + c0 + n_],
                                perf_mode=DR,
                            )
                            c0 += n_
                    pt = pt_pool.tile([P, 2, QTILE], BF16, tag="pt")
                    nc.scalar.activation(
                        pt[:, :, lo:],
                        psS[:, :, lo:],
                        ActF.Exp,
                        scale=float(SCALE),
                    )
                    if i >= 0:
                        # zero masked (k_local > q_local) on the diagonal
                        # 128-col block, both heads at once
                        if MASKDVE:
                            nc.vector.tensor_mul(
                                pt[:, :, lo : lo + P],
                                pt[:, :, lo : lo + P],
                                trimask[:],
                            )
                        else:
                            nc.gpsimd.affine_select(
                                out=pt[:, :, lo : lo + P],
                                in_=pt[:, :, lo : lo + P],
                                compare_op=mybir.AluOpType.is_ge,
                                fill=0.0,
                                base=0,
                                pattern=[[0, 2], [1, P]],
                                channel_multiplier=-1,
                            )
                    # software pipeline: S(kb) is emitted AVLAG k-blocks
                    # before AV(kb) so the in-order PE never stalls on exp
                    pending.append((kb, pt, lo))
                    if len(pending) > AVLAG:
                        emit_av(*pending.pop(0))
                    if filler is not None:
                        if next(filler, "END") == "END" and HEAT:
                            emit_dummy(b)
                for args in pending:
                    emit_av(*args)
                for h in range(2):
                    nc.vector.tensor_copy(
                        oSB[h][:, ts(qt, QTILE)], psO[h][:]
                    )

            # ---- prologue: batch 0 projections ----
            load_xt(0)
            vprime_alloc(0)
            for _ in window_proj(0):
                pass

            rep_ctx = (
                tc.For_i(0, reps, 1,
                         hint_engines=(mybir.EngineType.PE,
                                       mybir.EngineType.Activation,
                                       mybir.EngineType.DVE,
                                       mybir.EngineType.Pool,
                                       mybir.EngineType.SP))
                if reps > 1 else contextlib.nullcontext()
            )
            with rep_ctx:
              for _body_i in range(2 if BODY2 else 1):
                load_xt(1)
                vprime_alloc(1)
                oSB = [osb_pool.tile([HD + 1, T], F32, tag=f"oSB{h}",
                                     name=f"oSB{h}") for h in range(2)]
                filler = window_proj(1)
                for qt in range(NQT):
                    attn_qt(0, qt, oSB, filler)
                    if qt == 2:
                        out_q(out_d[0, 0, :, : 3 * QTILE],
                                            oSB[0][:, : 3 * QTILE])
                        out_q(out_d[0, 1, :, : 3 * QTILE],
                                            oSB[1][:, : 3 * QTILE])
                for _ in filler:
                    pass
                load_xt(0)
                out_q(out_d[0, 0, :, 3 * QTILE :],
                                    oSB[0][:, 3 * QTILE :])
                out_q(out_d[0, 1, :, 3 * QTILE :],
                                    oSB[1][:, 3 * QTILE :])
                vprime_alloc(0)
                oSB = [osb_pool.tile([HD + 1, T], F32, tag=f"oSB{h}",
                                     name=f"oSB{h}") for h in range(2)]
                filler = window_proj(0)
                for qt in range(NQT):
                    attn_qt(1, qt, oSB, filler)
                    if qt == 2:
                        out_q(out_d[1, 0, :, : 3 * QTILE],
                                            oSB[0][:, : 3 * QTILE])
                        out_q(out_d[1, 1, :, : 3 * QTILE],
                                            oSB[1][:, : 3 * QTILE])
                for _ in filler:
                    pass
                out_q(out_d[1, 0, :, 3 * QTILE :],
                                    oSB[0][:, 3 * QTILE :])
                out_q(out_d[1, 1, :, 3 * QTILE :],
                                    oSB[1][:, 3 * QTILE :])

    nc.compile()
    return nc


def _make_in_maps(inputs):
    x = np.ascontiguousarray(np.asarray(inputs["x"], np.float32))
    xt_f = np.ascontiguousarray(
        x.transpose(0, 2, 1).reshape(B, CB, P, T).transpose(0, 2, 1, 3)
    )
    xt = xt_f.astype(ml_dtypes.bfloat16)
    xt8 = xt_f.astype(ml_dtypes.float8_e4m3)
    Wq, Wk, Wv = inputs["Wq"], inputs["Wk"], inputs["Wv"]
    bq, bk, bv = inputs["bq"], inputs["bk"], inputs["bv"]

    in_maps = []
    for m in range(NCORES):
        sl = slice(m * P, (m + 1) * P)  # 128 output channels = 2 heads
        in_maps.append({
            "xt": xt,
            "xt8": xt8,
            "wq8": np.ascontiguousarray(
                np.asarray(Wq, np.float32)[sl, :].T
            ).astype(ml_dtypes.float8_e4m3),
            "wk8": np.ascontiguousarray(
                np.asarray(Wk, np.float32)[sl, :].T
            ).astype(ml_dtypes.float8_e4m3),
            "wqt": np.ascontiguousarray(
                np.asarray(Wq, np.float32)[sl, :].T
            ).astype(ml_dtypes.bfloat16),
            "wkt": np.ascontiguousarray(
                np.asarray(Wk, np.float32)[sl, :].T
            ).astype(ml_dtypes.bfloat16),
            "wvt": np.ascontiguousarray(
                np.asarray(Wv, np.float32)[sl, :].T
            ).astype(ml_dtypes.bfloat16),
            "bq": np.ascontiguousarray(np.asarray(bq, np.float32)[sl]),
            "bk": np.ascontiguousarray(np.asarray(bk, np.float32)[sl]),
            "bv": np.ascontiguousarray(np.asarray(bv, np.float32)[sl]),
        })
    return in_maps


_CACHED = {}


def kernel(x, Wq, bq, Wk, bk, Wv, bv):
    from concourse.bass_utils import run_bass_kernel_spmd

    if "nc" not in _CACHED:
        _CACHED["nc"] = _build_program()
    nc = _CACHED["nc"]

    in_maps = _make_in_maps(dict(
        x=x, Wq=Wq, bq=bq, Wk=Wk, bk=bk, Wv=Wv, bv=bv,
    ))

    res = run_bass_kernel_spmd(nc, in_maps, core_ids=list(range(NCORES)))

    parts = []
    for m in range(NCORES):
        o = res.results[m]["out"]  # [B, 2, 65, T] f32
        num = o[:, :, :HD, :]
        z = o[:, :, HD : HD + 1, :]
        core_out = (num / z).transpose(0, 3, 1, 2).reshape(B, T, 2 * HD)
        parts.append(core_out)
    return np.ascontiguousarray(np.concatenate(parts, axis=-1))
